# revision 15
# baseline (speedup 1.0000x reference)
"""Classical self-attention (head-summed scores) on 8 trn2 NeuronCores.

Math (per batch b):
    Q = x Wq; K = x Wk; V = x Wv          (W_qkv split columns 3x1024)
    S = Q K^T / 8   (full-E contraction: heads+dims summed)
    P = softmax(S, axis=-1)
    out = (P V) W_out + b_out

Sharding: 8 cores = (4 batches) x (2 query-halves). Each core gets its
batch's x rotated so its 1024 query rows come first; keys are the full
2048 rows (order of keys is irrelevant to the result). K/V projections
are duplicated between the 2 cores of a batch; no collectives needed.

Per-core kernel layout strategy:
  - S^T layout (keys on partitions) so the softmax reduction over keys
    becomes a ones-matmul and P^T feeds the O^T matmuls directly.
  - Softmax skips the max-subtraction (scores ~ N(0,4): exp stays well
    inside fp32 range); normalization by 1/rowsum is deferred to the
    final output projection where query rows sit on partitions.
  - All big matmuls in fp32r (tf32 datapath, full rate at free dim>=256).
  - K^T / Q^T / V staged through internal DRAM to stay under SBUF.
"""

import sys

sys.path.insert(0, "/opt/trn_rl_repo")

import numpy as np

import concourse.bass as bass
import concourse.mybir as mybir
import concourse.tile as tile
from concourse import bacc
from concourse.masks import make_identity

B, N, E = 4, 2048, 1024
NQ = N // 2          # query rows per core
P = 128              # partitions
FT = E // P          # 8 feature tiles (contraction for projections)
ET = E // P          # 8 embed tiles
MT = N // P          # 16 key tiles
QT = NQ // P         # 8 query tiles
MB = 4               # key tiles per projection block
NBLK = MT // MB      # 4 blocks
F32 = mybir.dt.float32
F32R = mybir.dt.float32r


def build_program():
    nc = bacc.Bacc("TRN2", target_bir_lowering=False, debug=False)
    x = nc.dram_tensor("x", [N, E], F32, kind="ExternalInput").ap()
    wqkv = nc.dram_tensor("wqkv", [E, 3 * E], F32, kind="ExternalInput").ap()
    wout = nc.dram_tensor("wout", [E, E], F32, kind="ExternalInput").ap()
    bout = nc.dram_tensor("bout", [E], F32, kind="ExternalInput").ap()
    y = nc.dram_tensor("y", [NQ, E], F32, kind="ExternalOutput").ap()

    with tile.TileContext(nc) as tc:
        _body(nc, tc, x, wqkv, wout, bout, y)
    nc.compile()
    return nc


def _body(nc, tc, x, wqkv, wout, bout, y):
    # DRAM staging (f32r so later loads need no cast)
    with tc.tile_pool(name="dram", bufs=1, space="DRAM") as dramp:
        kT_d = dramp.tile([E, N], F32R, name="kT_d", tag="kT_d")
        qT_d = dramp.tile([E, NQ], F32R, name="qT_d", tag="qT_d")
        v_d = dramp.tile([N, E], F32R, name="v_d", tag="v_d")

        _phase_project(nc, tc, x, wqkv, kT_d, qT_d, v_d)
        p_tiles, v_tiles, recip, pres, vrp, recp = _phase_scores(
            nc, tc, kT_d, qT_d, v_d)
        oT, oTp = _phase_pv(nc, tc, p_tiles, v_tiles, pres, vrp)
        _phase_out(nc, tc, oT, oTp, recip, wout, bout, y)
        oTp.release()
        recp.release()


def _phase_project(nc, tc, x, wqkv, kT_d, qT_d, v_d):
    """x -> x^T (PE transpose), then K^T, Q^T, V projections to DRAM."""
    with tc.tile_pool(name="wconst", bufs=1) as wcp, \
         tc.tile_pool(name="xin", bufs=3) as xp, \
         tc.tile_pool(name="xT", bufs=2) as xTp, \
         tc.tile_pool(name="ktmp", bufs=3) as ktp, \
         tc.tile_pool(name="vtmp", bufs=3) as vtp, \
         tc.tile_pool(name="tpps", bufs=2, space="PSUM") as tpp, \
         tc.tile_pool(name="pjps", bufs=4, space="PSUM") as pjp:

        ident = wcp.tile([P, P], F32, name="ident", tag="ident")
        make_identity(nc, ident)

        wq, wk, wv = [], [], []
        for f in range(FT):
            for lst, nm, c0 in ((wq, "wq", 0), (wk, "wk", E), (wv, "wv", 2 * E)):
                t = wcp.tile([P, E], F32R, name=f"{nm}{f}", tag=f"{nm}{f}")
                nc.gpsimd.dma_start(
                    out=t, in_=wqkv[f * P:(f + 1) * P, c0:c0 + E])
                lst.append(t)

        for blk in range(NBLK):
            xT = xTp.tile([P, FT, MB * P], F32R, name="xT", tag="xT")
            for mt in range(MB):
                m = blk * MB + mt
                xt = xp.tile([P, E], F32, name="xt", tag="xt")
                nc.sync.dma_start(out=xt, in_=x[m * P:(m + 1) * P, :])
                for f in range(FT):
                    tp = tpp.tile([P, P], F32, name="tp", tag="tp")
                    nc.tensor.transpose(tp, xt[:, f * P:(f + 1) * P], ident)
                    nc.any.tensor_copy(xT[:, f, mt * P:(mt + 1) * P], tp)

            # K^T block (all e rows, this block's key columns)
            for e in range(ET):
                ps = pjp.tile([P, MB * P], F32, name="pjk", tag="pj")
                for f in range(FT):
                    nc.tensor.matmul(ps, wk[f][:, e * P:(e + 1) * P],
                                     xT[:, f, :],
                                     start=(f == 0), stop=(f == FT - 1))
                kt_sb = ktp.tile([P, MB * P], F32R, name="kt_sb", tag="kt_sb")
                nc.any.tensor_copy(kt_sb, ps)
                nc.sync.dma_start(
                    out=kT_d[e * P:(e + 1) * P, blk * MB * P:(blk + 1) * MB * P],
                    in_=kt_sb)

            # Q^T block (first NQ rows are the queries)
            if blk * MB * P < NQ:
                for e in range(ET):
                    ps = pjp.tile([P, MB * P], F32, name="pjq", tag="pj")
                    for f in range(FT):
                        nc.tensor.matmul(ps, wq[f][:, e * P:(e + 1) * P],
                                         xT[:, f, :],
                                         start=(f == 0), stop=(f == FT - 1))
                    qt_sb = ktp.tile([P, MB * P], F32R, name="qt_sb", tag="kt_sb")
                    nc.any.tensor_copy(qt_sb, ps)
                    nc.sync.dma_start(
                        out=qT_d[e * P:(e + 1) * P,
                                 blk * MB * P:(blk + 1) * MB * P],
                        in_=qt_sb)

            # V block (natural layout rows)
            for mt in range(MB):
                m = blk * MB + mt
                vt = vtp.tile([P, E], F32R, name="vt", tag="vt")
                for h in range(2):
                    ps = pjp.tile([P, E // 2], F32, name="pjv", tag="pj")
                    for f in range(FT):
                        nc.tensor.matmul(
                            ps, xT[:, f, mt * P:(mt + 1) * P],
                            wv[f][:, h * (E // 2):(h + 1) * (E // 2)],
                            start=(f == 0), stop=(f == FT - 1))
                    nc.any.tensor_copy(vt[:, h * (E // 2):(h + 1) * (E // 2)], ps)
                nc.sync.dma_start(out=v_d[m * P:(m + 1) * P, :], in_=vt)


def _phase_scores(nc, tc, kT_d, qT_d, v_d):
    """S^T = K^T.T Q^T per key tile; P^T = exp(S^T/8); rowsums via ones-matmul.

    Also issues the V loads (consumed by phase_pv) so they overlap.
    Returns persistent P^T tiles, V tiles, and 1/rowsum [128, QT].
    """
    kT_r = kT_d.rearrange("(e p) m -> p e m", p=P)
    recp = tc.alloc_tile_pool(name="recp", bufs=1, side="right")
    pres = tc.alloc_tile_pool(name="pres", bufs=1)
    vrp = tc.alloc_tile_pool(name="vres", bufs=1)
    with tc.tile_pool(name="qT", bufs=1) as qTp, \
         tc.tile_pool(name="kts", bufs=2) as ktsp, \
         tc.tile_pool(name="small", bufs=1) as smp, \
         tc.tile_pool(name="sps", bufs=2, space="PSUM") as sp, \
         tc.tile_pool(name="sumps", bufs=2, space="PSUM") as sumsp:

        qT = []
        for e in range(ET):
            t = qTp.tile([P, NQ], F32R, name=f"qT{e}", tag=f"qT{e}")
            nc.sync.dma_start(out=t, in_=qT_d[e * P:(e + 1) * P, :])
            qT.append(t)

        v_tiles = []
        for m in range(MT):
            t = vrp.tile([P, E], F32R, name=f"v{m}", tag=f"v{m}")
            nc.sync.dma_start(out=t, in_=v_d[m * P:(m + 1) * P, :])
            v_tiles.append(t)

        ones = smp.tile([P, 1], F32, name="ones", tag="ones")
        nc.vector.memset(ones, 1.0)
        sums_acc = smp.tile([P, QT], F32, name="sums_acc", tag="sums_acc")

        p_tiles = []
        for m in range(MT):
            kt = ktsp.tile([P, ET, P], F32R, name="kt", tag="kt")
            nc.sync.dma_start(out=kt, in_=kT_r[:, :, m * P:(m + 1) * P])
            s = sp.tile([P, NQ], F32, name="s", tag="s")
            for e in range(ET):
                for h in range(2):
                    nc.tensor.matmul(
                        s[:, h * (NQ // 2):(h + 1) * (NQ // 2)],
                        kt[:, e, :],
                        qT[e][:, h * (NQ // 2):(h + 1) * (NQ // 2)],
                        start=(e == 0), stop=(e == ET - 1))
            p = pres.tile([P, NQ], F32R, name=f"p{m}", tag=f"p{m}")
            nc.scalar.activation(p, s, mybir.ActivationFunctionType.Exp,
                                 scale=0.125)
            sums_m = sumsp.tile([P, QT], F32, name="sums_m", tag="sums_m")
            for q in range(QT):
                nc.tensor.matmul(sums_m[:, q:q + 1],
                                 p[:, q * P:(q + 1) * P].bitcast(F32), ones,
                                 start=True, stop=True)
            if m == 0:
                nc.vector.tensor_copy(sums_acc, sums_m)
            else:
                nc.vector.tensor_tensor(out=sums_acc, in0=sums_acc,
                                        in1=sums_m, op=mybir.AluOpType.add)
            p_tiles.append(p)

        recip = recp.tile([P, QT], F32, name="recip", tag="recip")
        nc.vector.reciprocal(recip, sums_acc)

    return p_tiles, v_tiles, recip, pres, vrp, recp


def _phase_pv(nc, tc, p_tiles, v_tiles, pres, vrp):
    """O^T[e, nq] = sum_m V[m,e]^T P^T[m,nq], accumulated in PSUM."""
    oTp = tc.alloc_tile_pool(name="oTp", bufs=1, side="right")
    oT = [oTp.tile([P, NQ], F32R, name=f"oT{e}", tag=f"oT{e}")
          for e in range(ET)]
    H = NQ // 2
    with tc.tile_pool(name="ops", bufs=1, space="PSUM") as opp:
        for half in range(2):
            o_ps = [opp.tile([P, H], F32, name=f"o{e}", tag=f"o{e}")
                    for e in range(ET)]
            for m in range(MT):
                for e in range(ET):
                    nc.tensor.matmul(
                        o_ps[e], v_tiles[m][:, e * P:(e + 1) * P],
                        p_tiles[m][:, half * H:(half + 1) * H],
                        start=(m == 0), stop=(m == MT - 1))
            for e in range(ET):
                nc.any.tensor_copy(oT[e][:, half * H:(half + 1) * H], o_ps[e])
    vrp.release()
    pres.release()
    return oT, oTp


def _phase_out(nc, tc, oT, oTp, recip, wout, bout, y):
    """y rows = (O_u W_out) * recip + b_out."""
    with tc.tile_pool(name="wo", bufs=1) as wop, \
         tc.tile_pool(name="ysb", bufs=3) as ysp, \
         tc.tile_pool(name="yps", bufs=2, space="PSUM") as ypp:

        wo = []
        for e in range(ET):
            t = wop.tile([P, E], F32R, name=f"wo{e}", tag=f"wo{e}")
            nc.gpsimd.dma_start(out=t, in_=wout[e * P:(e + 1) * P, :])
            wo.append(t)
        bo_b = wop.tile([P, E], F32, name="bo_b", tag="bo_b")
        bout_bcast = bass.AP(tensor=bout.tensor, offset=0,
                             ap=[[0, P], [1, E]])
        nc.sync.dma_start(out=bo_b, in_=bout_bcast)

        H = E // 2
        for nqt in range(QT):
            yps = ypp.tile([P, E], F32, name="yps", tag="yps")
            for e in range(ET):
                for h in range(2):
                    nc.tensor.matmul(
                        yps[:, h * H:(h + 1) * H],
                        oT[e][:, nqt * P:(nqt + 1) * P],
                        wo[e][:, h * H:(h + 1) * H],
                        start=(e == 0), stop=(e == ET - 1))
            ysb = ysp.tile([P, E], F32, name="ysb", tag="ysb")
            nc.vector.tensor_scalar_mul(ysb, yps, recip[:, nqt:nqt + 1])
            nc.vector.tensor_tensor(out=ysb, in0=ysb, in1=bo_b,
                                    op=mybir.AluOpType.add)
            nc.sync.dma_start(out=y[nqt * P:(nqt + 1) * P, :], in_=ysb)


_NC_CACHE = None


def _get_program():
    global _NC_CACHE
    if _NC_CACHE is None:
        _NC_CACHE = build_program()
    return _NC_CACHE


def kernel(x, W_qkv, W_out, b_out):
    from concourse.bass_utils import run_bass_kernel_spmd

    x = np.asarray(x, dtype=np.float32)
    W_qkv = np.asarray(W_qkv, dtype=np.float32)
    W_out = np.asarray(W_out, dtype=np.float32)
    b_out = np.asarray(b_out, dtype=np.float32)

    nc = _get_program()
    in_maps = []
    for c in range(8):
        b, half = divmod(c, 2)
        xb = x[b]
        xrot = np.ascontiguousarray(
            np.concatenate([xb[half * NQ:], xb[:half * NQ]], axis=0))
        in_maps.append({"x": xrot, "wqkv": W_qkv, "wout": W_out,
                       "bout": b_out})
    res = run_bass_kernel_spmd(nc, in_maps, list(range(8)))
    out = np.empty((B, N, E), dtype=np.float32)
    for c in range(8):
        b, half = divmod(c, 2)
        out[b, half * NQ:(half + 1) * NQ] = res.results[c]["y"]
    return out


# revision 16
# speedup vs baseline: 1.0232x; 1.0232x over previous
"""Classical self-attention (head-summed scores) on 8 trn2 NeuronCores.

Math (per batch b):
    Q = x Wq; K = x Wk; V = x Wv          (W_qkv split columns 3x1024)
    S = Q K^T / 8   (full-E contraction: heads+dims summed)
    P = softmax(S, axis=-1)
    out = (P V) W_out + b_out

Sharding: 8 cores = (4 batches) x (2 query-halves). Each core gets its
batch's x rotated so its 1024 query rows come first; keys are the full
2048 rows (key order is irrelevant to the result). K/V projections are
duplicated between the 2 cores of a batch; no collectives needed.

Per-core kernel layout strategy:
  - S^T layout (keys on partitions) so the softmax reduction over keys
    becomes a ones-matmul and P^T feeds the O^T matmuls directly.
  - Softmax skips the max-subtraction (scores ~ N(0,4): exp stays well
    inside fp32 range); normalization by 1/rowsum is deferred to the
    final output projection where query rows sit on partitions.
  - All big matmuls in fp32r (tf32 datapath, full rate at free dim>=256).
  - K^T and V staged through internal DRAM to stay under SBUF; Q^T stays
    SBUF-resident so the scores phase overlaps the projection phase.
"""

import sys

sys.path.insert(0, "/opt/trn_rl_repo")

import numpy as np

import concourse.bass as bass
import concourse.mybir as mybir
import concourse.tile as tile
from concourse import bacc
from concourse.masks import make_identity

B, N, E = 4, 2048, 1024
NQ = N // 2          # query rows per core
P = 128              # partitions
FT = E // P          # 8 feature tiles (contraction for projections)
ET = E // P          # 8 embed tiles
MT = N // P          # 16 key tiles
QT = NQ // P         # 8 query tiles
MB = 4               # key tiles per projection block
NBLK = MT // MB      # 4 blocks
F32 = mybir.dt.float32
F32R = mybir.dt.float32r


def build_program():
    nc = bacc.Bacc("TRN2", target_bir_lowering=False, debug=False)
    x = nc.dram_tensor("x", [N, E], F32, kind="ExternalInput").ap()
    wqkv = nc.dram_tensor("wqkv", [E, 3 * E], F32, kind="ExternalInput").ap()
    wout = nc.dram_tensor("wout", [E, E], F32, kind="ExternalInput").ap()
    bout = nc.dram_tensor("bout", [E], F32, kind="ExternalInput").ap()
    y = nc.dram_tensor("y", [NQ, E], F32, kind="ExternalOutput").ap()

    with tile.TileContext(nc) as tc:
        _body(nc, tc, x, wqkv, wout, bout, y)
    nc.compile()
    return nc


def _body(nc, tc, x, wqkv, wout, bout, y):
    with tc.tile_pool(name="dram", bufs=1, space="DRAM") as dramp:
        kT_d = dramp.tile([E, N], F32R, name="kT_d", tag="kT_d")
        v_d = dramp.tile([N, E], F32R, name="v_d", tag="v_d")

        qTp = tc.alloc_tile_pool(name="qTp", bufs=1)
        qT = [qTp.tile([P, NQ], F32R, name=f"qT{e}", tag=f"qT{e}")
              for e in range(ET)]

        _phase_project(nc, tc, x, wqkv, kT_d, v_d, qT)
        p_tiles, recip, pres, recp = _phase_scores(nc, tc, kT_d, qT)
        oT, oTp = _phase_pv(nc, tc, p_tiles, v_d, pres, qTp)
        _phase_out(nc, tc, oT, recip, wout, bout, y)
        oTp.release()
        recp.release()


def _phase_project(nc, tc, x, wqkv, kT_d, v_d, qT):
    """x -> x^T (PE transpose), then K^T (to DRAM), Q^T (SBUF), V (DRAM)."""
    with tc.tile_pool(name="wconst", bufs=1) as wcp, \
         tc.tile_pool(name="xin", bufs=3) as xp, \
         tc.tile_pool(name="xT", bufs=2) as xTp, \
         tc.tile_pool(name="ktmp", bufs=2) as ktp, \
         tc.tile_pool(name="vtmp", bufs=2) as vtp, \
         tc.tile_pool(name="tpps", bufs=2, space="PSUM") as tpp, \
         tc.tile_pool(name="pjps", bufs=4, space="PSUM") as pjp:

        ident = wcp.tile([P, P], F32, name="ident", tag="ident")
        make_identity(nc, ident)

        # Wk first: the first projection matmuls need it soonest.
        wk, wq, wv = [], [], []
        for lst, nm, c0 in ((wk, "wk", E), (wq, "wq", 0), (wv, "wv", 2 * E)):
            for f in range(FT):
                t = wcp.tile([P, E], F32R, name=f"{nm}{f}", tag=f"{nm}{f}")
                nc.gpsimd.dma_start(
                    out=t, in_=wqkv[f * P:(f + 1) * P, c0:c0 + E])
                lst.append(t)

        for blk in range(NBLK):
            xT = xTp.tile([P, FT, MB * P], F32R, name="xT", tag="xT")
            for mt in range(MB):
                m = blk * MB + mt
                xt = xp.tile([P, E], F32, name="xt", tag="xt")
                nc.sync.dma_start(out=xt, in_=x[m * P:(m + 1) * P, :])
                for f in range(FT):
                    tp = tpp.tile([P, P], F32, name="tp", tag="tp")
                    nc.tensor.transpose(tp, xt[:, f * P:(f + 1) * P], ident)
                    nc.vector.tensor_copy(xT[:, f, mt * P:(mt + 1) * P], tp)

            # K^T block (all e rows, this block's key columns)
            for e in range(ET):
                ps = pjp.tile([P, MB * P], F32, name="pjk", tag="pj")
                for f in range(FT):
                    nc.tensor.matmul(ps, wk[f][:, e * P:(e + 1) * P],
                                     xT[:, f, :],
                                     start=(f == 0), stop=(f == FT - 1))
                kt_sb = ktp.tile([P, MB * P], F32R, name="kt_sb", tag="kt_sb")
                nc.vector.tensor_copy(kt_sb, ps)
                nc.sync.dma_start(
                    out=kT_d[e * P:(e + 1) * P, blk * MB * P:(blk + 1) * MB * P],
                    in_=kt_sb)

            # Q^T block straight into resident SBUF tiles
            if blk * MB * P < NQ:
                for e in range(ET):
                    ps = pjp.tile([P, MB * P], F32, name="pjq", tag="pj")
                    for f in range(FT):
                        nc.tensor.matmul(ps, wq[f][:, e * P:(e + 1) * P],
                                         xT[:, f, :],
                                         start=(f == 0), stop=(f == FT - 1))
                    nc.vector.tensor_copy(
                        qT[e][:, blk * MB * P:(blk + 1) * MB * P], ps)

            # V block (natural layout rows) to DRAM
            for mt in range(MB):
                m = blk * MB + mt
                vt = vtp.tile([P, E], F32R, name="vt", tag="vt")
                for h in range(2):
                    ps = pjp.tile([P, E // 2], F32, name="pjv", tag="pj")
                    for f in range(FT):
                        nc.tensor.matmul(
                            ps, xT[:, f, mt * P:(mt + 1) * P],
                            wv[f][:, h * (E // 2):(h + 1) * (E // 2)],
                            start=(f == 0), stop=(f == FT - 1))
                    nc.vector.tensor_copy(
                        vt[:, h * (E // 2):(h + 1) * (E // 2)], ps)
                nc.sync.dma_start(out=v_d[m * P:(m + 1) * P, :], in_=vt)


def _phase_scores(nc, tc, kT_d, qT):
    """S^T = K^T.T Q^T per key tile; P^T = exp(S^T/8); rowsums via ones-matmul."""
    kT_r = kT_d.rearrange("(e p) m -> p e m", p=P)
    recp = tc.alloc_tile_pool(name="recp", bufs=1, side="right")
    pres = tc.alloc_tile_pool(name="pres", bufs=1)
    with tc.tile_pool(name="kts", bufs=2) as ktsp, \
         tc.tile_pool(name="small", bufs=1) as smp, \
         tc.tile_pool(name="sps", bufs=2, space="PSUM") as sp, \
         tc.tile_pool(name="sumps", bufs=2, space="PSUM") as sumsp:

        ones = smp.tile([P, 1], F32, name="ones", tag="ones")
        nc.vector.memset(ones, 1.0)
        sums_acc = smp.tile([P, QT], F32, name="sums_acc", tag="sums_acc")

        p_tiles = []
        for m in range(MT):
            kt = ktsp.tile([P, ET, P], F32R, name="kt", tag="kt")
            nc.sync.dma_start(out=kt, in_=kT_r[:, :, m * P:(m + 1) * P])
            s = sp.tile([P, NQ], F32, name="s", tag="s")
            for e in range(ET):
                for h in range(2):
                    nc.tensor.matmul(
                        s[:, h * (NQ // 2):(h + 1) * (NQ // 2)],
                        kt[:, e, :],
                        qT[e][:, h * (NQ // 2):(h + 1) * (NQ // 2)],
                        start=(e == 0), stop=(e == ET - 1))
            p = pres.tile([P, NQ], F32R, name=f"p{m}", tag=f"p{m}")
            nc.scalar.activation(p, s, mybir.ActivationFunctionType.Exp,
                                 scale=0.125)
            sums_m = sumsp.tile([P, QT], F32, name="sums_m", tag="sums_m")
            for q in range(QT):
                nc.tensor.matmul(sums_m[:, q:q + 1],
                                 p[:, q * P:(q + 1) * P].bitcast(F32), ones,
                                 start=True, stop=True)
            if m == 0:
                nc.vector.tensor_copy(sums_acc, sums_m)
            else:
                nc.vector.tensor_tensor(out=sums_acc, in0=sums_acc,
                                        in1=sums_m, op=mybir.AluOpType.add)
            p_tiles.append(p)

        recip = recp.tile([P, QT], F32, name="recip", tag="recip")
        nc.vector.reciprocal(recip, sums_acc)

    return p_tiles, recip, pres, recp


def _phase_pv(nc, tc, p_tiles, v_d, pres, qTp):
    """O^T[e, nq] = sum_m V[m,e]^T P^T[m,nq], accumulated in PSUM.

    V tiles are streamed from DRAM (twice: once per nq half).
    """
    oTp = tc.alloc_tile_pool(name="oTp", bufs=1, side="right")
    oT = [oTp.tile([P, NQ], F32R, name=f"oT{e}", tag=f"oT{e}")
          for e in range(ET)]
    H = NQ // 2
    with tc.tile_pool(name="vstream", bufs=4) as vsp, \
         tc.tile_pool(name="ops", bufs=1, space="PSUM") as opp:
        for half in range(2):
            o_ps = [opp.tile([P, H], F32, name=f"o{e}", tag=f"o{e}")
                    for e in range(ET)]
            for m in range(MT):
                vt = vsp.tile([P, E], F32R, name="vs", tag="vs")
                nc.sync.dma_start(out=vt, in_=v_d[m * P:(m + 1) * P, :])
                for e in range(ET):
                    nc.tensor.matmul(
                        o_ps[e], vt[:, e * P:(e + 1) * P],
                        p_tiles[m][:, half * H:(half + 1) * H],
                        start=(m == 0), stop=(m == MT - 1))
            for e in range(ET):
                nc.vector.tensor_copy(oT[e][:, half * H:(half + 1) * H],
                                      o_ps[e])
    pres.release()
    qTp.release()
    return oT, oTp


def _phase_out(nc, tc, oT, recip, wout, bout, y):
    """y rows = (O_u W_out) * recip + b_out."""
    with tc.tile_pool(name="wo", bufs=1) as wop, \
         tc.tile_pool(name="ysb", bufs=3) as ysp, \
         tc.tile_pool(name="yps", bufs=2, space="PSUM") as ypp:

        wo = []
        for e in range(ET):
            t = wop.tile([P, E], F32R, name=f"wo{e}", tag=f"wo{e}")
            nc.gpsimd.dma_start(out=t, in_=wout[e * P:(e + 1) * P, :])
            wo.append(t)
        bo_b = wop.tile([P, E], F32, name="bo_b", tag="bo_b")
        bout_bcast = bass.AP(tensor=bout.tensor, offset=0,
                             ap=[[0, P], [1, E]])
        nc.sync.dma_start(out=bo_b, in_=bout_bcast)

        H = E // 2
        for nqt in range(QT):
            yps = ypp.tile([P, E], F32, name="yps", tag="yps")
            for e in range(ET):
                for h in range(2):
                    nc.tensor.matmul(
                        yps[:, h * H:(h + 1) * H],
                        oT[e][:, nqt * P:(nqt + 1) * P],
                        wo[e][:, h * H:(h + 1) * H],
                        start=(e == 0), stop=(e == ET - 1))
            ysb = ysp.tile([P, E], F32, name="ysb", tag="ysb")
            nc.vector.tensor_scalar_mul(ysb, yps, recip[:, nqt:nqt + 1])
            nc.vector.tensor_tensor(out=ysb, in0=ysb, in1=bo_b,
                                    op=mybir.AluOpType.add)
            nc.sync.dma_start(out=y[nqt * P:(nqt + 1) * P, :], in_=ysb)


_NC_CACHE = None


def _get_program():
    global _NC_CACHE
    if _NC_CACHE is None:
        _NC_CACHE = build_program()
    return _NC_CACHE


def kernel(x, W_qkv, W_out, b_out):
    from concourse.bass_utils import run_bass_kernel_spmd

    x = np.asarray(x, dtype=np.float32)
    W_qkv = np.asarray(W_qkv, dtype=np.float32)
    W_out = np.asarray(W_out, dtype=np.float32)
    b_out = np.asarray(b_out, dtype=np.float32)

    nc = _get_program()
    in_maps = []
    for c in range(8):
        b, half = divmod(c, 2)
        xb = x[b]
        xrot = np.ascontiguousarray(
            np.concatenate([xb[half * NQ:], xb[:half * NQ]], axis=0))
        in_maps.append({"x": xrot, "wqkv": W_qkv, "wout": W_out,
                       "bout": b_out})
    res = run_bass_kernel_spmd(nc, in_maps, list(range(8)))
    out = np.empty((B, N, E), dtype=np.float32)
    for c in range(8):
        b, half = divmod(c, 2)
        out[b, half * NQ:(half + 1) * NQ] = res.results[c]["y"]
    return out


# revision 38
# speedup vs baseline: 1.0275x; 1.0042x over previous
"""Classical self-attention (head-summed scores) on 8 trn2 NeuronCores.

Math (per batch b):
    Q = x Wq; K = x Wk; V = x Wv          (W_qkv split columns 3x1024)
    S = Q K^T / 8   (full-E contraction: heads+dims summed)
    P = softmax(S, axis=-1)
    out = (P V) W_out + b_out

Sharding: 8 cores = (4 batches) x (2 query-halves). Each core gets its
batch's x rotated so its 1024 query rows come first; keys are the full
2048 rows (key order is irrelevant to the result). K/V projections are
duplicated between the 2 cores of a batch; no collectives needed.

Per-core kernel layout strategy:
  - S^T layout (keys on partitions) so the softmax reduction over keys
    becomes a ones-matmul and P^T feeds the O^T matmuls directly.
  - Softmax skips the max-subtraction (scores ~ N(0,4): exp stays well
    inside fp32 range); normalization by 1/rowsum is deferred to the
    final output projection where query rows sit on partitions.
  - All big matmuls in fp32r (tf32 datapath, full rate at free dim>=256).
  - K^T and V staged through internal DRAM to stay under SBUF; Q^T stays
    SBUF-resident so the scores phase overlaps the projection phase.
"""

import sys

sys.path.insert(0, "/opt/trn_rl_repo")

import numpy as np

import concourse.bass as bass
import concourse.mybir as mybir
import concourse.tile as tile
from concourse import bacc
from concourse.masks import make_identity

B, N, E = 4, 2048, 1024
NQ = N // 2          # query rows per core
P = 128              # partitions
FT = E // P          # 8 feature tiles (contraction for projections)
ET = E // P          # 8 embed tiles
MT = N // P          # 16 key tiles
QT = NQ // P         # 8 query tiles
MB = 4               # key tiles per projection block
NBLK = MT // MB      # 4 blocks
F32 = mybir.dt.float32
F32R = mybir.dt.float32r


def build_program():
    nc = bacc.Bacc("TRN2", target_bir_lowering=False, debug=False)
    x = nc.dram_tensor("x", [N, E], F32, kind="ExternalInput").ap()
    wqkv = nc.dram_tensor("wqkv", [E, 3 * E], F32, kind="ExternalInput").ap()
    wout = nc.dram_tensor("wout", [E, E], F32, kind="ExternalInput").ap()
    bout = nc.dram_tensor("bout", [E], F32, kind="ExternalInput").ap()
    y = nc.dram_tensor("y", [NQ, E], F32, kind="ExternalOutput").ap()

    with tile.TileContext(nc) as tc:
        _body(nc, tc, x, wqkv, wout, bout, y)
    nc.compile()
    return nc


def _body(nc, tc, x, wqkv, wout, bout, y):
    with tc.tile_pool(name="dram", bufs=1, space="DRAM") as dramp:
        kT_d = dramp.tile([E, N], F32R, name="kT_d", tag="kT_d")
        v_d = dramp.tile([N, E], F32R, name="v_d", tag="v_d")

        qTp = tc.alloc_tile_pool(name="qTp", bufs=1)
        qT = [qTp.tile([P, NQ], F32R, name=f"qT{e}", tag=f"qT{e}")
              for e in range(ET)]

        _phase_project(nc, tc, x, wqkv, kT_d, v_d, qT)

        # W_out / b_out tiles; DMAs issued at phase_scores start.
        wop = tc.alloc_tile_pool(name="wo", bufs=1)
        wo = [wop.tile([P, E], F32R, name=f"wo{e}", tag=f"wo{e}")
              for e in range(ET)]
        bo_b = wop.tile([P, E], F32, name="bo_b", tag="bo_b")
        bout_bcast = bass.AP(tensor=bout.tensor, offset=0,
                             ap=[[0, P], [1, E]])
        for e in range(ET):
            nc.gpsimd.dma_start(out=wo[e], in_=wout[e * P:(e + 1) * P, :])
        nc.sync.dma_start(out=bo_b, in_=bout_bcast)

        p_tiles, recip, pres, recp = _phase_scores(nc, tc, kT_d, qT, [])
        oT, oTp = _phase_pv(nc, tc, p_tiles, v_d, pres)
        _phase_out(nc, tc, oT, recip, wo, bo_b, y)
        wop.release()
        qTp.release()
        oTp.release()
        recp.release()


def _phase_project(nc, tc, x, wqkv, kT_d, v_d, qT):
    """x -> x^T (PE transpose), then K^T (to DRAM), Q^T (SBUF), V (DRAM)."""
    with tc.tile_pool(name="wconst", bufs=1) as wcp, \
         tc.tile_pool(name="xin", bufs=3) as xp, \
         tc.tile_pool(name="xT", bufs=2) as xTp, \
         tc.tile_pool(name="ktmp", bufs=2) as ktp, \
         tc.tile_pool(name="vtmp", bufs=2) as vtp, \
         tc.tile_pool(name="tpps", bufs=2, space="PSUM") as tpp, \
         tc.tile_pool(name="pjps", bufs=4, space="PSUM") as pjp:

        ident = wcp.tile([P, P], F32, name="ident", tag="ident")
        make_identity(nc, ident)

        # Wk first: the first projection matmuls need it soonest.
        wk, wq, wv = [], [], []
        for lst, nm, c0 in ((wk, "wk", E), (wq, "wq", 0), (wv, "wv", 2 * E)):
            for f in range(FT):
                t = wcp.tile([P, E], F32R, name=f"{nm}{f}", tag=f"{nm}{f}")
                nc.gpsimd.dma_start(
                    out=t, in_=wqkv[f * P:(f + 1) * P, c0:c0 + E])
                lst.append(t)

        for blk in range(NBLK):
            xT = xTp.tile([P, FT, MB * P], F32R, name="xT", tag="xT")
            for mt in range(MB):
                m = blk * MB + mt
                xt = xp.tile([P, E], F32, name="xt", tag="xt")
                nc.sync.dma_start(out=xt, in_=x[m * P:(m + 1) * P, :])
                for f in range(FT):
                    tp = tpp.tile([P, P], F32, name="tp", tag="tp")
                    nc.tensor.transpose(tp, xt[:, f * P:(f + 1) * P], ident)
                    nc.vector.tensor_copy(xT[:, f, mt * P:(mt + 1) * P], tp)

            # K^T block (all e rows, this block's key columns)
            for e in range(ET):
                ps = pjp.tile([P, MB * P], F32, name="pjk", tag="pj")
                for f in range(FT):
                    nc.tensor.matmul(ps, wk[f][:, e * P:(e + 1) * P],
                                     xT[:, f, :],
                                     start=(f == 0), stop=(f == FT - 1))
                kt_sb = ktp.tile([P, MB * P], F32R, name="kt_sb", tag="kt_sb")
                nc.vector.tensor_copy(kt_sb, ps)
                nc.sync.dma_start(
                    out=kT_d[e * P:(e + 1) * P, blk * MB * P:(blk + 1) * MB * P],
                    in_=kt_sb)

            # Q^T block straight into resident SBUF tiles
            if blk * MB * P < NQ:
                for e in range(ET):
                    ps = pjp.tile([P, MB * P], F32, name="pjq", tag="pj")
                    for f in range(FT):
                        nc.tensor.matmul(ps, wq[f][:, e * P:(e + 1) * P],
                                         xT[:, f, :],
                                         start=(f == 0), stop=(f == FT - 1))
                    nc.vector.tensor_copy(
                        qT[e][:, blk * MB * P:(blk + 1) * MB * P], ps)

            # V block (natural layout rows) to DRAM
            for mt in range(MB):
                m = blk * MB + mt
                vt = vtp.tile([P, E], F32R, name="vt", tag="vt")
                for h in range(2):
                    ps = pjp.tile([P, E // 2], F32, name="pjv", tag="pj")
                    for f in range(FT):
                        nc.tensor.matmul(
                            ps, xT[:, f, mt * P:(mt + 1) * P],
                            wv[f][:, h * (E // 2):(h + 1) * (E // 2)],
                            start=(f == 0), stop=(f == FT - 1))
                    nc.vector.tensor_copy(
                        vt[:, h * (E // 2):(h + 1) * (E // 2)], ps)
                nc.sync.dma_start(out=v_d[m * P:(m + 1) * P, :], in_=vt)


def _phase_scores(nc, tc, kT_d, qT, wo_loads):
    """S^T = K^T.T Q^T per key tile; P^T = exp(S^T/8); rowsums via ones-matmul."""
    kT_r = kT_d.rearrange("(e p) m -> p e m", p=P)
    recp = tc.alloc_tile_pool(name="recp", bufs=1, side="right")
    pres = tc.alloc_tile_pool(name="pres", bufs=1)
    with tc.tile_pool(name="kts", bufs=3) as ktsp, \
         tc.tile_pool(name="small", bufs=1) as smp, \
         tc.tile_pool(name="sps", bufs=3, space="PSUM") as sp, \
         tc.tile_pool(name="sumps", bufs=2, space="PSUM") as sumsp:

        ones = smp.tile([P, 1], F32, name="ones", tag="ones")
        nc.vector.memset(ones, 1.0)
        sums_acc = smp.tile([P, QT], F32, name="sums_acc", tag="sums_acc")

        p_tiles = []
        for m in range(MT):
            kt = ktsp.tile([P, ET, P], F32R, name="kt", tag="kt")
            nc.sync.dma_start(out=kt, in_=kT_r[:, :, m * P:(m + 1) * P])
            s = sp.tile([P, NQ], F32, name="s", tag="s")
            for e in range(ET):
                for h in range(2):
                    nc.tensor.matmul(
                        s[:, h * (NQ // 2):(h + 1) * (NQ // 2)],
                        kt[:, e, :],
                        qT[e][:, h * (NQ // 2):(h + 1) * (NQ // 2)],
                        start=(e == 0), stop=(e == ET - 1))
            p = pres.tile([P, NQ], F32R, name=f"p{m}", tag=f"p{m}")
            nc.scalar.activation(p, s, mybir.ActivationFunctionType.Exp,
                                 scale=0.125)
            p_tiles.append(p)
            # Row-sum the PREVIOUS tile's exp: its activation ran while
            # this tile's S matmuls were on PE, so PE never waits on ACT.
            if m > 0:
                _row_sums(nc, p_tiles[m - 1], sumsp, smp, ones, sums_acc,
                          first=(m == 1))
        _row_sums(nc, p_tiles[MT - 1], sumsp, smp, ones, sums_acc,
                  first=False)

        recip = recp.tile([P, QT], F32, name="recip", tag="recip")
        nc.vector.reciprocal(recip, sums_acc)

    return p_tiles, recip, pres, recp


def _row_sums(nc, p, sumsp, smp, ones, sums_acc, first):
    sums_m = sumsp.tile([P, QT], F32, name="sums_m", tag="sums_m")
    for q in range(QT):
        nc.tensor.matmul(sums_m[:, q:q + 1],
                         p[:, q * P:(q + 1) * P].bitcast(F32), ones,
                         start=True, stop=True)
    if first:
        nc.vector.tensor_copy(sums_acc, sums_m)
    else:
        nc.vector.tensor_tensor(out=sums_acc, in0=sums_acc,
                                in1=sums_m, op=mybir.AluOpType.add)


def _phase_pv(nc, tc, p_tiles, v_d, pres):
    """O^T[e, nq] = sum_m V[m,e]^T P^T[m,nq], accumulated in PSUM.

    e-tiles are processed in 2 groups of 4 so each group's O^T rows fit
    in PSUM ([128, NQ] x 4 = 8 banks) and V streams from DRAM only once
    per group (half its columns each time).
    """
    oTp = tc.alloc_tile_pool(name="oTp", bufs=1, side="right")
    oT = [oTp.tile([P, NQ], F32R, name=f"oT{e}", tag=f"oT{e}")
          for e in range(ET)]
    EG = ET // 2
    H = NQ // 2
    with tc.tile_pool(name="vstream", bufs=4) as vsp, \
         tc.tile_pool(name="ops", bufs=1, space="PSUM") as opp:
        for g in range(2):
            o_ps = [opp.tile([P, NQ], F32, name=f"o{j}", tag=f"o{j}")
                    for j in range(EG)]
            for m in range(MT):
                vt = vsp.tile([P, EG * P], F32R, name="vs", tag="vs")
                nc.sync.dma_start(
                    out=vt,
                    in_=v_d[m * P:(m + 1) * P, g * EG * P:(g + 1) * EG * P])
                for j in range(EG):
                    for h in range(2):
                        nc.tensor.matmul(
                            o_ps[j][:, h * H:(h + 1) * H],
                            vt[:, j * P:(j + 1) * P],
                            p_tiles[m][:, h * H:(h + 1) * H],
                            start=(m == 0), stop=(m == MT - 1))
            for j in range(EG):
                nc.vector.tensor_copy(oT[g * EG + j], o_ps[j])
    pres.release()
    return oT, oTp


def _phase_out(nc, tc, oT, recip, wo, bo_b, y):
    """y rows = (O_u W_out) * recip + b_out."""
    with tc.tile_pool(name="ysb", bufs=3) as ysp, \
         tc.tile_pool(name="yps", bufs=2, space="PSUM") as ypp:

        H = E // 2
        for nqt in range(QT):
            yps = ypp.tile([P, E], F32, name="yps", tag="yps")
            for e in range(ET):
                for h in range(2):
                    nc.tensor.matmul(
                        yps[:, h * H:(h + 1) * H],
                        oT[e][:, nqt * P:(nqt + 1) * P],
                        wo[e][:, h * H:(h + 1) * H],
                        start=(e == 0), stop=(e == ET - 1))
            ysb = ysp.tile([P, E], F32, name="ysb", tag="ysb")
            nc.vector.tensor_scalar_mul(ysb, yps, recip[:, nqt:nqt + 1])
            nc.vector.tensor_tensor(out=ysb, in0=ysb, in1=bo_b,
                                    op=mybir.AluOpType.add)
            nc.sync.dma_start(out=y[nqt * P:(nqt + 1) * P, :], in_=ysb)


_NC_CACHE = None


def _get_program():
    global _NC_CACHE
    if _NC_CACHE is None:
        _NC_CACHE = build_program()
    return _NC_CACHE


def kernel(x, W_qkv, W_out, b_out):
    from concourse.bass_utils import run_bass_kernel_spmd

    x = np.asarray(x, dtype=np.float32)
    W_qkv = np.asarray(W_qkv, dtype=np.float32)
    W_out = np.asarray(W_out, dtype=np.float32)
    b_out = np.asarray(b_out, dtype=np.float32)

    nc = _get_program()
    in_maps = []
    for c in range(8):
        b, half = divmod(c, 2)
        xb = x[b]
        xrot = np.ascontiguousarray(
            np.concatenate([xb[half * NQ:], xb[:half * NQ]], axis=0))
        in_maps.append({"x": xrot, "wqkv": W_qkv, "wout": W_out,
                       "bout": b_out})
    res = run_bass_kernel_spmd(nc, in_maps, list(range(8)))
    out = np.empty((B, N, E), dtype=np.float32)
    for c in range(8):
        b, half = divmod(c, 2)
        out[b, half * NQ:(half + 1) * NQ] = res.results[c]["y"]
    return out
